# revision 7
# baseline (speedup 1.0000x reference)
"""Block-causal (anti-causal: key-block >= query-block) multi-head attention
for Trainium2, run SPMD on 8 NeuronCores.

Problem (hardcoded): B=2, T=8, N=256 (L=2048), D=768, H=12, HD=64.
reference:
    qkv = x @ qkv_w.T + qkv_b ; split into q,k,v heads
    s   = (q @ k.T) / 8 ; mask: query in block ti attends keys in blocks tj >= ti
    p   = softmax(s) ; y = p @ v ; out = y @ proj_w.T + proj_b

Sharding: data-parallel over B (2) x tensor-parallel over heads (4 groups of
3 heads) = 8 cores. Each core computes, for its (batch, head-group):
  - QKV^T   = Wsel @ x^T  (bf16 matmuls; bias folded into the PSUM->SBUF
              copy as a DVE tensor_scalar add)
  - S^T     = K^T.T-chunks vs Q^T   (keys on partitions, queries on free dim)
  - P~      = exp(0.125 * S^T)      (no max-subtraction; logits are tiny)
  - U^T     = [V|1].T @ P~           (ones-column gives softmax denominator row)
  - O^T     = U^T * (1/den) broadcast
  - Z^T    += Wproj-slice @ O^T      (partial projection output, bf16)
Host sums the 4 head-group partials per batch and adds proj_b.

Schedule: attention is processed per query-quarter (qq outer, head inner).
Within a (head, qq) group, key chunks are paired and software-pipelined:
the S matmuls for pair k+1 are emitted before the PV matmuls for pair k,
so the scalar-engine exp overlaps PE work instead of serializing the
S -> exp -> PV chain.  Key chunks are reordered so the first PV matmul of
each group covers the full 512-query PSUM bank with start=True (the
pending-zero region is bank-granular).  The projection for quarter qq is
emitted during the attention of quarter qq+1 so the output DMA overlaps
compute.
"""

import functools

import ml_dtypes
import numpy as np

import concourse.bass as bass
import concourse.bacc as bacc_mod
import concourse.mybir as mybir
import concourse.tile as tile
from concourse.bass import ts

F32 = mybir.dt.float32
BF16 = mybir.dt.bfloat16

B, T, N, D = 2, 8, 256, 768
H, HD = 12, 64
L = T * N          # 2048
HPC = 3            # heads per core
NKC = L // 128     # 16 key chunks of 128
NDC = D // 128     # 6 contraction chunks
SCALE = 1.0 / 8.0


def build_nc():
    nc = bacc_mod.Bacc()

    xT_d = nc.declare_dram_parameter("xT", [D, L], BF16, isOutput=False)
    wqkvT_d = nc.declare_dram_parameter("wqkvT", [D, 576], BF16, isOutput=False)
    bqkv_d = nc.declare_dram_parameter("bqkv", [128, 8], F32, isOutput=False)
    aux_d = nc.declare_dram_parameter("aux", [128, 80], BF16, isOutput=False)
    wprojT_d = nc.declare_dram_parameter("wprojT", [128, 1536], BF16, isOutput=False)
    zT_d = nc.declare_dram_parameter("zT", [D, L], BF16, isOutput=True)

    with tile.TileContext(nc) as tc:
        with (
            tc.tile_pool(name="persist", bufs=1) as pp,
            tc.tile_pool(name="ptile", bufs=4) as ppool,
            tc.tile_pool(name="zbuf", bufs=3) as zpool,
            tc.tile_pool(name="psum_st", bufs=2, space="PSUM") as pst,
            tc.tile_pool(name="psum_ot", bufs=2, space="PSUM") as pot,
            tc.tile_pool(name="psum_mc", bufs=2, space="PSUM") as pmc,
        ):
            # ---- persistent SBUF tensors ----
            wqkvT = pp.tile([128, NDC, 576], BF16, tag="wqkvT")
            bq = pp.tile([128, 8], F32, tag="bq")
            aux = pp.tile([128, 80], BF16, tag="aux")
            wprojT = pp.tile([128, 1536], BF16, tag="wprojT")
            # qkv-transposed activations: rows are head dims
            qt = pp.tile([128, L], BF16, tag="qt")      # [q_h0 | q_h1]
            kt = pp.tile([128, L], BF16, tag="kt")      # [k_h0 | k_h1]
            vt = pp.tile([128, L], BF16, tag="vt")      # [v_h0 | v_h1]
            qk2 = pp.tile([128, L], BF16, tag="qk2")    # [q_h2 | k_h2]
            kt2 = pp.tile([64, L], BF16, tag="kt2")     # k_h2 re-based to partition 0
            vt2 = pp.tile([64, L], BF16, tag="vt2")     # [v_h2]
            # natural-layout V per head, augmented with a ones column
            vn = [
                pp.tile([128, NKC, 65], BF16, tag=f"vn{h}", name=f"vn{h}")
                for h in range(HPC)
            ]
            # normalized attention outputs (transposed): rows are head dims
            otp = pp.tile([128, L], BF16, tag="otp")    # [o_h0 | o_h1]
            ots = pp.tile([64, L], BF16, tag="ots")     # [o_h2]
            bcast = pp.tile([64, 512], F32, tag="bcast")
            den = pp.tile([1, 512], F32, tag="den")

            # ---- input DMAs: weights first, then x, wproj last ----
            for dc in range(NDC):
                nc.sync.dma_start(
                    out=wqkvT[:, dc, :], in_=wqkvT_d[ts(dc, 128), :]
                )
            nc.sync.dma_start(out=bq[:], in_=bqkv_d[:, :])
            nc.sync.dma_start(out=aux[:], in_=aux_d[:, :])
            with tc.tile_pool(name="xT", bufs=1) as xp:
                xT = xp.tile([128, NDC, L], BF16, tag="xT")
                for half in range(2):
                    for dc in range(NDC):
                        nc.sync.dma_start(
                            out=xT[:, dc, ts(half, 1024)],
                            in_=xT_d[ts(dc, 128), ts(half, 1024)],
                        )
                nc.sync.dma_start(out=wprojT[:], in_=wprojT_d[:, :])

                for h in range(HPC):
                    nc.vector.tensor_copy(vn[h][:, :, 64], aux[:, 64:80])
                nc.vector.memset(bcast[:], 1.0)
                # Pre-warm the exp activation table during the qkv phase.
                warm = zpool.tile([128, 32], F32, tag="warm")
                nc.vector.memset(warm[:], 0.0)
                nc.scalar.activation(
                    warm[:], warm[:], mybir.ActivationFunctionType.Exp
                )

                # ---- phase 1: QKV^T = Wsel @ x^T; bias in the copy ----
                # M-chunks of the 576 output dims (order fixed host-side):
                # 0:[q0|q1] 1:[k0|k1] 2:[v0|v1] 3:[q2|k2] 4:[v2] (64 rows)
                mc_dst = [qt, kt, vt, qk2, vt2]
                vt_src = [vt[0:64, :], vt[64:128, :], vt2[0:64, :]]
                id_src = [aux[0:64, 0:64], aux[64:128, 0:64], aux[0:64, 0:64]]
                for nt in range(4):
                    for mc in range(5):
                        mrows = 64 if mc == 4 else 128
                        ps = pmc.tile([128, 512], F32, tag="qs")
                        for dc in range(NDC):
                            nc.tensor.matmul(
                                ps[0:mrows, :],
                                wqkvT[:, dc, mc * 128 : mc * 128 + mrows],
                                xT[:, dc, ts(nt, 512)],
                                start=(dc == 0),
                                stop=(dc == NDC - 1),
                            )
                        nc.vector.tensor_scalar_add(
                            mc_dst[mc][0:mrows, ts(nt, 512)],
                            ps[0:mrows, :],
                            bq[0:mrows, mc : mc + 1],
                        )
                    # V natural layout via PE transpose for this nt's chunks
                    for kc in range(4 * nt, 4 * nt + 4):
                        for h in range(HPC):
                            tp = pot.tile([128, 64], BF16, tag="ot")
                            nc.tensor.transpose(
                                tp[:, 0:64], vt_src[h][:, ts(kc, 128)], id_src[h]
                            )
                            nc.vector.tensor_copy(vn[h][:, kc, 0:64], tp[:, 0:64])

            # ---- attention ----
            # k_h2 sits at partitions 64:128 of qk2 while q_h2 is at 0:64; the
            # PE needs both matmul operands on the same partitions, so re-base
            # k_h2 with an SBUF->SBUF DMA (DMA is partition-agnostic).
            nc.gpsimd.dma_start(out=kt2[0:64, :], in_=qk2[64:128, :])
            qt_src = [qt[0:64, :], qt[64:128, :], qk2[0:64, :]]
            kt_src = [kt[0:64, :], kt[64:128, :], kt2[0:64, :]]
            ot_dst = [otp[0:64, :], otp[64:128, :], ots[0:64, :]]
            SHUF_ID0 = [0] * 32

            def emit_pv(h, ot, job):
                pi, a, b, pt, masked = job
                if not masked:
                    nc.tensor.matmul(
                        ot[0:65, 0:512], vn[h][:, a, :], pt[:, 0:512],
                        start=(pi == 0), stop=False, skip_group_check=True,
                    )
                    nc.tensor.matmul(
                        ot[0:65, 0:512], vn[h][:, b, :], pt[:, 512:1024],
                        start=False, stop=False, skip_group_check=True,
                    )
                else:
                    nc.tensor.matmul(
                        ot[0:65, 0:256], vn[h][:, a, :], pt[:, 0:256],
                        start=False, stop=False, skip_group_check=True,
                    )
                    nc.tensor.matmul(
                        ot[0:65, 0:256], vn[h][:, b, :], pt[:, 512:768],
                        start=False, stop=True, skip_group_check=True,
                    )

            def attn_group(h, qq):
                q_lo = qq * 512
                # Masked pair (kb == qb0: only the first 256 queries attend)
                # goes LAST so the group's first PV covers the full bank with
                # start=True.
                kcs = list(range(4 * qq + 2, 16)) + [4 * qq, 4 * qq + 1]
                pairs = [(kcs[i], kcs[i + 1]) for i in range(0, len(kcs), 2)]
                ot = pot.tile([128, 512], F32, tag="ot")
                pending = []
                for pi, (a, b) in enumerate(pairs):
                    masked = a == 4 * qq
                    seg = 256 if masked else 512
                    # B always goes to the second PSUM bank: two start=True
                    # groups must not share a 2KB bank (zero region).
                    off_b = 512
                    st2 = pst.tile([128, 1024], F32, tag="st")
                    nc.tensor.matmul(
                        st2[:, 0:seg],
                        kt_src[h][:, ts(a, 128)],
                        qt_src[h][:, q_lo : q_lo + seg],
                        start=True, stop=True,
                    )
                    nc.tensor.matmul(
                        st2[:, off_b : off_b + seg],
                        kt_src[h][:, ts(b, 128)],
                        qt_src[h][:, q_lo : q_lo + seg],
                        start=True, stop=True,
                    )
                    pt = ppool.tile([128, 1024], BF16, tag="pt")
                    nc.scalar.activation(
                        pt[:, 0 : off_b + seg],
                        st2[:, 0 : off_b + seg],
                        mybir.ActivationFunctionType.Exp,
                        scale=SCALE,
                    )
                    pending.append((pi, a, b, pt, masked))
                    if len(pending) > 1:
                        emit_pv(h, ot, pending.pop(0))
                emit_pv(h, ot, pending.pop(0))
                # normalize: inv = 1/den broadcast across 64 partitions (DVE).
                # den goes via SBUF: custom-DVE reciprocal from PSUM is
                # untrusted on HW.
                nc.vector.tensor_copy(den[0:1, :], ot[64:65, 0:512])
                nc.vector.reciprocal_approx_fast(bcast[0:1, :], den[0:1, :])
                nc.vector.stream_shuffle(bcast[0:32, :], bcast[0:32, :], SHUF_ID0)
                nc.vector.stream_shuffle(bcast[32:64, :], bcast[0:32, :], SHUF_ID0)
                nc.vector.tensor_tensor(
                    out=ot_dst[h][:, q_lo : q_lo + 512],
                    in0=ot[0:64, 0:512],
                    in1=bcast[0:64, :],
                    op=mybir.AluOpType.mult,
                )

            def proj(qq):
                for mc in range(NDC):
                    ps = pmc.tile([128, 512], F32, tag="qs")
                    nc.tensor.matmul(
                        ps[:],
                        wprojT[:, ts(mc, 128)],
                        otp[:, ts(qq, 512)],
                        start=True, stop=False,
                    )
                    nc.tensor.matmul(
                        ps[:],
                        wprojT[0:64, 768 + mc * 128 : 768 + (mc + 1) * 128],
                        ots[0:64, ts(qq, 512)],
                        start=False, stop=True,
                    )
                    zb = zpool.tile([128, 512], BF16, tag="zb")
                    nc.vector.tensor_copy(zb[:], ps[:])
                    nc.sync.dma_start(
                        out=zT_d[ts(mc, 128), ts(qq, 512)], in_=zb[:]
                    )

            for qq in range(4):
                for h in range(HPC):
                    attn_group(h, qq)
                    if h == 0 and qq > 0:
                        proj(qq - 1)
            proj(3)

    nc.compile()
    return nc


@functools.lru_cache(maxsize=1)
def get_nc():
    return build_nc()


def make_in_maps(x, qkv_w, qkv_b, proj_w):
    """Per-core host-side sharding/layout prep."""
    x = np.asarray(x, dtype=np.float32)
    qkv_w = np.asarray(qkv_w, dtype=np.float32)
    qkv_b = np.asarray(qkv_b, dtype=np.float32)
    proj_w = np.asarray(proj_w, dtype=np.float32)

    in_maps = []
    for c in range(8):
        b, g = divmod(c, 4)
        h0, h1, h2 = 3 * g, 3 * g + 1, 3 * g + 2

        def qrows(h):
            return slice(h * HD, (h + 1) * HD)

        def krows(h):
            return slice(D + h * HD, D + (h + 1) * HD)

        def vrows(h):
            return slice(2 * D + h * HD, 2 * D + (h + 1) * HD)

        order = [
            qrows(h0), qrows(h1), krows(h0), krows(h1), vrows(h0), vrows(h1),
            qrows(h2), krows(h2), vrows(h2),
        ]
        wsel = np.concatenate([qkv_w[s] for s in order], axis=0)      # (576, 768)
        bsel = np.concatenate([qkv_b[s] for s in order], axis=0)      # (576,)
        bcol = np.zeros((128, 8), np.float32)
        for mc in range(5):
            rows = 64 if mc == 4 else 128
            bcol[:rows, mc] = bsel[mc * 128 : mc * 128 + rows]
        wpp = np.concatenate(
            [proj_w[:, ts_np(h0)].T, proj_w[:, ts_np(h1)].T], axis=0
        )  # (128, 768)
        wps = np.concatenate(
            [proj_w[:, ts_np(h2)].T, np.zeros((64, D), np.float32)], axis=0
        )  # (128, 768)
        in_maps.append(
            {
                "xT": np.ascontiguousarray(x[b].reshape(L, D).T).astype(
                    ml_dtypes.bfloat16
                ),
                "wqkvT": np.ascontiguousarray(wsel.T).astype(ml_dtypes.bfloat16),
                "bqkv": bcol,
                "aux": AUX.astype(ml_dtypes.bfloat16),
                "wprojT": np.ascontiguousarray(
                    np.concatenate([wpp, wps], axis=1)
                ).astype(ml_dtypes.bfloat16),
            }
        )
    return in_maps


AUX = np.concatenate(
    [
        np.concatenate([np.eye(64, dtype=np.float32)] * 2, axis=0),
        np.ones((128, 16), np.float32),
    ],
    axis=1,
)


def ts_np(h):
    return slice(h * HD, (h + 1) * HD)


def assemble_output(results, proj_b):
    proj_b = np.asarray(proj_b, dtype=np.float32)
    out = np.zeros((B, L, D), np.float32)
    for c in range(8):
        b = c // 4
        out[b] += results[c]["zT"].astype(np.float32).T
    out += proj_b[None, None, :]
    return out.reshape(B, T, N, D)


def _install_ntff_hook():
    """The container's antenv stub lacks axon_hooks; recreate it from the
    boot helper so trace=True can profile through libaxon_pjrt."""
    import sys
    import types

    try:
        from antenv.axon_hooks import get_axon_ntff_profile_hook  # noqa: F401

        return
    except ImportError:
        pass
    import antenv
    from trn_agent_boot.trn_boot import _ntff_profile_via_ctypes

    state = {"hook": _ntff_profile_via_ctypes("/opt/axon/libaxon_pjrt.so")}
    mod = types.ModuleType("antenv.axon_hooks")
    mod.set_axon_ntff_profile_hook = lambda h: state.__setitem__("hook", h)
    mod.get_axon_ntff_profile_hook = lambda: state["hook"]
    sys.modules["antenv.axon_hooks"] = mod
    antenv.axon_hooks = mod

    import concourse.bass_utils as bu

    orig_upload = bu.upload_artifacts

    def safe_upload(tmpdir):
        try:
            return orig_upload(tmpdir)
        except Exception:
            return tmpdir

    bu.upload_artifacts = safe_upload


def kernel_with_stats(x, qkv_w, qkv_b, proj_w, proj_b, trace=False):
    from concourse.bass_utils import run_bass_kernel_spmd

    if trace:
        _install_ntff_hook()
    nc = get_nc()
    in_maps = make_in_maps(x, qkv_w, qkv_b, proj_w)
    res = run_bass_kernel_spmd(nc, in_maps, list(range(8)), trace=trace)
    return assemble_output(res.results, proj_b), res


def kernel(x, qkv_w, qkv_b, proj_w, proj_b):
    out, _ = kernel_with_stats(x, qkv_w, qkv_b, proj_w, proj_b)
    return out


# revision 11
# speedup vs baseline: 1.0986x; 1.0986x over previous
"""Block-causal (anti-causal: key-block >= query-block) multi-head attention
for Trainium2, run SPMD on 8 NeuronCores.

Problem (hardcoded): B=2, T=8, N=256 (L=2048), D=768, H=12, HD=64.
reference:
    qkv = x @ qkv_w.T + qkv_b ; split into q,k,v heads
    s   = (q @ k.T) / 8 ; mask: query in block ti attends keys in blocks tj >= ti
    p   = softmax(s) ; y = p @ v ; out = y @ proj_w.T + proj_b

Sharding: data-parallel over B (2) x tensor-parallel over heads (4 groups of
3 heads) = 8 cores. Each core computes, for its (batch, head-group):
  - Q^T,K^T = Wqk @ x^T   (bf16 matmuls; bias folded into the PSUM->SBUF
              copy on the scalar engine: Identity activation + bias AP)
  - Vn      = x-chunk.T @ Wv-stream: V in NATURAL layout [keys, 3*(64+1)]
              per 128-key chunk (xT chunk stationary, V weights moving).
              The per-head ones column (softmax denominator) is a zero
              weight column plus 1.0 in the broadcast bias added on DVE.
  - S^T     = K^T.T-chunks vs Q^T   (keys on partitions, queries on free dim)
  - P~      = exp(0.125 * S^T)      (no max-subtraction; logits are tiny)
  - U^T     = Vn.T @ P~              (ones column gives denominator row)
  - O^T     = U^T * (1/den) broadcast
  - Z^T    += Wproj-slice @ O^T      (partial projection output, bf16)
Host sums the 4 head-group partials per batch and adds proj_b.

Schedule: attention is processed per query-quarter (qq outer, head inner).
Within a (head, qq) group, key chunks are paired and software-pipelined:
the S matmuls for pair k+1 are emitted before the PV matmuls for pair k,
so the scalar-engine exp overlaps PE work instead of serializing the
S -> exp -> PV chain.  Key chunks are reordered so the first PV matmul of
each group covers the full 512-query PSUM bank with start=True (the
pending-zero region is bank-granular).  The projection for quarter qq is
emitted during the attention of quarter qq+1 so the output DMA overlaps
compute.
"""

import functools

import ml_dtypes
import numpy as np

import concourse.bass as bass
import concourse.bacc as bacc_mod
import concourse.mybir as mybir
import concourse.tile as tile
from concourse.bass import ts

F32 = mybir.dt.float32
BF16 = mybir.dt.bfloat16

B, T, N, D = 2, 8, 256, 768
H, HD = 12, 64
L = T * N          # 2048
HPC = 3            # heads per core
NKC = L // 128     # 16 key chunks of 128
NDC = D // 128     # 6 contraction chunks
SCALE = 1.0 / 8.0
VW = 3 * 65        # natural-V row width: [v_h0 | 1] [v_h1 | 1] [v_h2 | 1]


def build_nc():
    nc = bacc_mod.Bacc()

    xT_d = nc.declare_dram_parameter("xT", [D, L], BF16, isOutput=False)
    wqkT_d = nc.declare_dram_parameter("wqkT", [D, 384], BF16, isOutput=False)
    wvT3_d = nc.declare_dram_parameter("wvT3", [D, VW], BF16, isOutput=False)
    bqk_d = nc.declare_dram_parameter("bqk", [128, 4], F32, isOutput=False)
    bvb_d = nc.declare_dram_parameter("bvb", [128, VW], F32, isOutput=False)
    wprojT_d = nc.declare_dram_parameter("wprojT", [128, 1536], BF16, isOutput=False)
    zT_d = nc.declare_dram_parameter("zT", [D, L], BF16, isOutput=True)

    with tile.TileContext(nc) as tc:
        with (
            tc.tile_pool(name="persist", bufs=1) as pp,
            tc.tile_pool(name="ptile", bufs=4) as ppool,
            tc.tile_pool(name="zbuf", bufs=3) as zpool,
            tc.tile_pool(name="psum_st", bufs=2, space="PSUM") as pst,
            tc.tile_pool(name="psum_ot", bufs=2, space="PSUM") as pot,
            tc.tile_pool(name="psum_mc", bufs=2, space="PSUM") as pmc,
        ):
            # ---- persistent SBUF tensors ----
            wqkT = pp.tile([128, NDC, 384], BF16, tag="wqkT")
            wvT3 = pp.tile([128, NDC, VW], BF16, tag="wvT3")
            bqk = pp.tile([128, 4], F32, tag="bqk")
            bvb = pp.tile([128, VW], F32, tag="bvb")
            wprojT = pp.tile([128, 1536], BF16, tag="wprojT")
            # qk-transposed activations: rows are head dims
            qt = pp.tile([128, L], BF16, tag="qt")      # [q_h0 | q_h1]
            kt = pp.tile([128, L], BF16, tag="kt")      # [k_h0 | k_h1]
            qk2 = pp.tile([128, L], BF16, tag="qk2")    # [q_h2 | k_h2]
            kt2 = pp.tile([64, L], BF16, tag="kt2")     # k_h2 re-based to partition 0
            # natural-layout V, all heads: [keys, kc, VW]
            vn = pp.tile([128, NKC, VW], BF16, tag="vn")
            # normalized attention outputs (transposed): rows are head dims
            otp = pp.tile([128, L], BF16, tag="otp")    # [o_h0 | o_h1]
            ots = pp.tile([64, L], BF16, tag="ots")     # [o_h2]
            bcast = pp.tile([64, 512], F32, tag="bcast")
            den = pp.tile([1, 512], F32, tag="den")

            # ---- input DMAs, split across two issue engines ----
            # sync: the QKV-phase critical path (qk weights, then x chunks);
            # gpsimd: everything needed later.
            nc.sync.dma_start(
                out=wqkT[:],
                in_=wqkT_d[:, :].rearrange("(dc p) w -> p dc w", p=128),
            )
            with tc.tile_pool(name="xT", bufs=1) as xp:
                xT = xp.tile([128, NDC, L], BF16, tag="xT")
                for nt in range(4):
                    nc.sync.dma_start(
                        out=xT[:, :, ts(nt, 512)],
                        in_=xT_d[:, ts(nt, 512)].rearrange(
                            "(dc p) w -> p dc w", p=128
                        ),
                    )
                nc.gpsimd.dma_start(out=bqk[:], in_=bqk_d[:, :])
                nc.gpsimd.dma_start(
                    out=wvT3[:],
                    in_=wvT3_d[:, :].rearrange("(dc p) w -> p dc w", p=128),
                )
                nc.gpsimd.dma_start(out=bvb[:], in_=bvb_d[:, :])
                nc.gpsimd.dma_start(out=wprojT[:], in_=wprojT_d[:, :])
                nc.vector.memset(bcast[:], 1.0)

                # ---- phase 1: Q^T/K^T chains + natural-V chains ----
                # qk M-chunks: 0:[q0|q1] 1:[k0|k1] 2:[q2|k2]
                mc_dst = [qt, kt, qk2]
                for nt in range(4):
                    for mc in range(3):
                        ps = pmc.tile([128, 512], F32, tag="qs")
                        for dc in range(NDC):
                            nc.tensor.matmul(
                                ps[:],
                                wqkT[:, dc, ts(mc, 128)],
                                xT[:, dc, ts(nt, 512)],
                                start=(dc == 0),
                                stop=(dc == NDC - 1),
                            )
                        nc.scalar.activation(
                            mc_dst[mc][:, ts(nt, 512)],
                            ps[:],
                            mybir.ActivationFunctionType.Identity,
                            bias=bqk[:, mc : mc + 1],
                        )
                    # natural-layout V for this nt's key chunks: x stationary,
                    # V weights moving; ones columns are zero-weight + bias.
                    for kc in range(4 * nt, 4 * nt + 4):
                        vp = pot.tile([128, 256], F32, tag="ot")
                        for dc in range(NDC):
                            nc.tensor.matmul(
                                vp[:, 0:VW],
                                xT[:, dc, ts(kc, 128)],
                                wvT3[:, dc, :],
                                start=(dc == 0),
                                stop=(dc == NDC - 1),
                            )
                        nc.vector.tensor_tensor(
                            out=vn[:, kc, :],
                            in0=vp[:, 0:VW],
                            in1=bvb[:],
                            op=mybir.AluOpType.add,
                        )
                # Pre-warm the exp table after the Identity bias-copies.
                warm = zpool.tile([128, 32], F32, tag="warm")
                nc.vector.memset(warm[:], 0.0)
                nc.scalar.activation(
                    warm[:], warm[:], mybir.ActivationFunctionType.Exp
                )

            # ---- attention ----
            # k_h2 sits at partitions 64:128 of qk2 while q_h2 is at 0:64; the
            # PE needs both matmul operands on the same partitions, so re-base
            # k_h2 with an SBUF->SBUF DMA (DMA is partition-agnostic).
            nc.gpsimd.dma_start(out=kt2[0:64, :], in_=qk2[64:128, :])
            qt_src = [qt[0:64, :], qt[64:128, :], qk2[0:64, :]]
            kt_src = [kt[0:64, :], kt[64:128, :], kt2[0:64, :]]
            ot_dst = [otp[0:64, :], otp[64:128, :], ots[0:64, :]]
            SHUF_ID0 = [0] * 32

            def emit_pv(h, ot, job):
                pi, a, b, pt, masked = job
                va = vn[:, a, 65 * h : 65 * h + 65]
                vb = vn[:, b, 65 * h : 65 * h + 65]
                if not masked:
                    nc.tensor.matmul(
                        ot[0:65, 0:512], va, pt[:, 0:512],
                        start=(pi == 0), stop=False, skip_group_check=True,
                    )
                    nc.tensor.matmul(
                        ot[0:65, 0:512], vb, pt[:, 512:1024],
                        start=False, stop=False, skip_group_check=True,
                    )
                else:
                    nc.tensor.matmul(
                        ot[0:65, 0:256], va, pt[:, 0:256],
                        start=False, stop=False, skip_group_check=True,
                    )
                    nc.tensor.matmul(
                        ot[0:65, 0:256], vb, pt[:, 512:768],
                        start=False, stop=True, skip_group_check=True,
                    )

            def attn_group(h, qq):
                q_lo = qq * 512
                # Masked pair (kb == qb0: only the first 256 queries attend)
                # goes LAST so the group's first PV covers the full bank with
                # start=True.
                kcs = list(range(4 * qq + 2, 16)) + [4 * qq, 4 * qq + 1]
                pairs = [(kcs[i], kcs[i + 1]) for i in range(0, len(kcs), 2)]
                ot = pot.tile([128, 512], F32, tag="ot")
                pending = []
                for pi, (a, b) in enumerate(pairs):
                    masked = a == 4 * qq
                    seg = 256 if masked else 512
                    # B always goes to the second PSUM bank: two start=True
                    # groups must not share a 2KB bank (zero region).
                    st2 = pst.tile([128, 1024], F32, tag="st")
                    nc.tensor.matmul(
                        st2[:, 0:seg],
                        kt_src[h][:, ts(a, 128)],
                        qt_src[h][:, q_lo : q_lo + seg],
                        start=True, stop=True,
                    )
                    nc.tensor.matmul(
                        st2[:, 512 : 512 + seg],
                        kt_src[h][:, ts(b, 128)],
                        qt_src[h][:, q_lo : q_lo + seg],
                        start=True, stop=True,
                    )
                    pt = ppool.tile([128, 1024], BF16, tag="pt")
                    if masked:
                        # Two ACTs over the written regions only (cols
                        # 256:512 of st2 were never written).
                        for off in (0, 512):
                            nc.scalar.activation(
                                pt[:, off : off + 256],
                                st2[:, off : off + 256],
                                mybir.ActivationFunctionType.Exp,
                                scale=SCALE,
                            )
                    else:
                        nc.scalar.activation(
                            pt[:, 0:1024],
                            st2[:, 0:1024],
                            mybir.ActivationFunctionType.Exp,
                            scale=SCALE,
                        )
                    pending.append((pi, a, b, pt, masked))
                    if len(pending) > 1:
                        emit_pv(h, ot, pending.pop(0))
                emit_pv(h, ot, pending.pop(0))
                # normalize: inv = 1/den broadcast across 64 partitions (DVE).
                # den goes via SBUF: custom-DVE reciprocal from PSUM is
                # untrusted on HW.
                nc.vector.tensor_copy(den[0:1, :], ot[64:65, 0:512])
                nc.vector.reciprocal_approx_fast(bcast[0:1, :], den[0:1, :])
                nc.vector.stream_shuffle(bcast[0:32, :], bcast[0:32, :], SHUF_ID0)
                nc.vector.stream_shuffle(bcast[32:64, :], bcast[0:32, :], SHUF_ID0)
                nc.vector.tensor_tensor(
                    out=ot_dst[h][:, q_lo : q_lo + 512],
                    in0=ot[0:64, 0:512],
                    in1=bcast[0:64, :],
                    op=mybir.AluOpType.mult,
                )

            def proj(qq, casts_on_scalar=False):
                for mc in range(NDC):
                    ps = pmc.tile([128, 512], F32, tag="qs")
                    nc.tensor.matmul(
                        ps[:],
                        wprojT[:, ts(mc, 128)],
                        otp[:, ts(qq, 512)],
                        start=True, stop=False,
                    )
                    nc.tensor.matmul(
                        ps[:],
                        wprojT[0:64, 768 + mc * 128 : 768 + (mc + 1) * 128],
                        ots[0:64, ts(qq, 512)],
                        start=False, stop=True,
                    )
                    zb = zpool.tile([128, 512], BF16, tag="zb")
                    if casts_on_scalar:
                        nc.scalar.copy(zb[:], ps[:])
                    else:
                        nc.vector.tensor_copy(zb[:], ps[:])
                    nc.sync.dma_start(
                        out=zT_d[ts(mc, 128), ts(qq, 512)], in_=zb[:]
                    )

            for qq in range(4):
                for h in range(HPC):
                    attn_group(h, qq)
                    if h == 0 and qq > 0:
                        proj(qq - 1)
            proj(3, casts_on_scalar=True)

    nc.compile()
    return nc


@functools.lru_cache(maxsize=1)
def get_nc():
    return build_nc()


def make_in_maps(x, qkv_w, qkv_b, proj_w):
    """Per-core host-side sharding/layout prep."""
    x = np.asarray(x, dtype=np.float32)
    qkv_w = np.asarray(qkv_w, dtype=np.float32)
    qkv_b = np.asarray(qkv_b, dtype=np.float32)
    proj_w = np.asarray(proj_w, dtype=np.float32)

    in_maps = []
    for c in range(8):
        b, g = divmod(c, 4)
        h0, h1, h2 = 3 * g, 3 * g + 1, 3 * g + 2

        def qrows(h):
            return slice(h * HD, (h + 1) * HD)

        def krows(h):
            return slice(D + h * HD, D + (h + 1) * HD)

        def vrows(h):
            return slice(2 * D + h * HD, 2 * D + (h + 1) * HD)

        # qk selection: mc0=[q0|q1] mc1=[k0|k1] mc2=[q2|k2]
        order = [
            qrows(h0), qrows(h1), krows(h0), krows(h1), qrows(h2), krows(h2),
        ]
        wqk = np.concatenate([qkv_w[s] for s in order], axis=0)       # (384, 768)
        bqk_sel = np.concatenate([qkv_b[s] for s in order], axis=0)   # (384,)
        bcol = np.zeros((128, 4), np.float32)
        for mc in range(3):
            bcol[:, mc] = bqk_sel[mc * 128 : (mc + 1) * 128]
        # natural-V weights: per head 64 cols + one zero col (ones slot)
        wv3 = np.zeros((VW, D), np.float32)
        bv3 = np.zeros(VW, np.float32)
        for i, h in enumerate((h0, h1, h2)):
            wv3[65 * i : 65 * i + 64] = qkv_w[vrows(h)]
            bv3[65 * i : 65 * i + 64] = qkv_b[vrows(h)]
            bv3[65 * i + 64] = 1.0
        wpp = np.concatenate(
            [proj_w[:, ts_np(h0)].T, proj_w[:, ts_np(h1)].T], axis=0
        )  # (128, 768)
        wps = np.concatenate(
            [proj_w[:, ts_np(h2)].T, np.zeros((64, D), np.float32)], axis=0
        )  # (128, 768)
        in_maps.append(
            {
                "xT": np.ascontiguousarray(x[b].reshape(L, D).T).astype(
                    ml_dtypes.bfloat16
                ),
                "wqkT": np.ascontiguousarray(wqk.T).astype(ml_dtypes.bfloat16),
                "wvT3": np.ascontiguousarray(wv3.T).astype(ml_dtypes.bfloat16),
                "bqk": bcol,
                "bvb": np.broadcast_to(bv3, (128, VW)).copy(),
                "wprojT": np.ascontiguousarray(
                    np.concatenate([wpp, wps], axis=1)
                ).astype(ml_dtypes.bfloat16),
            }
        )
    return in_maps


def ts_np(h):
    return slice(h * HD, (h + 1) * HD)


def assemble_output(results, proj_b):
    proj_b = np.asarray(proj_b, dtype=np.float32)
    out = np.zeros((B, L, D), np.float32)
    for c in range(8):
        b = c // 4
        out[b] += results[c]["zT"].astype(np.float32).T
    out += proj_b[None, None, :]
    return out.reshape(B, T, N, D)


def _install_ntff_hook():
    """The container's antenv stub lacks axon_hooks; recreate it from the
    boot helper so trace=True can profile through libaxon_pjrt."""
    import sys
    import types

    try:
        from antenv.axon_hooks import get_axon_ntff_profile_hook  # noqa: F401

        return
    except ImportError:
        pass
    import antenv
    from trn_agent_boot.trn_boot import _ntff_profile_via_ctypes

    state = {"hook": _ntff_profile_via_ctypes("/opt/axon/libaxon_pjrt.so")}
    mod = types.ModuleType("antenv.axon_hooks")
    mod.set_axon_ntff_profile_hook = lambda h: state.__setitem__("hook", h)
    mod.get_axon_ntff_profile_hook = lambda: state["hook"]
    sys.modules["antenv.axon_hooks"] = mod
    antenv.axon_hooks = mod

    import concourse.bass_utils as bu

    orig_upload = bu.upload_artifacts

    def safe_upload(tmpdir):
        try:
            return orig_upload(tmpdir)
        except Exception:
            return tmpdir

    bu.upload_artifacts = safe_upload


def kernel_with_stats(x, qkv_w, qkv_b, proj_w, proj_b, trace=False):
    from concourse.bass_utils import run_bass_kernel_spmd

    if trace:
        _install_ntff_hook()
    nc = get_nc()
    in_maps = make_in_maps(x, qkv_w, qkv_b, proj_w)
    res = run_bass_kernel_spmd(nc, in_maps, list(range(8)), trace=trace)
    return assemble_output(res.results, proj_b), res


def kernel(x, qkv_w, qkv_b, proj_w, proj_b):
    out, _ = kernel_with_stats(x, qkv_w, qkv_b, proj_w, proj_b)
    return out


# revision 12
# speedup vs baseline: 1.1330x; 1.0314x over previous
"""Block-causal (anti-causal: key-block >= query-block) multi-head attention
for Trainium2, run SPMD on 8 NeuronCores.

Problem (hardcoded): B=2, T=8, N=256 (L=2048), D=768, H=12, HD=64.
reference:
    qkv = x @ qkv_w.T + qkv_b ; split into q,k,v heads
    s   = (q @ k.T) / 8 ; mask: query in block ti attends keys in blocks tj >= ti
    p   = softmax(s) ; y = p @ v ; out = y @ proj_w.T + proj_b

Sharding: data-parallel over B (2) x tensor-parallel over heads (4 groups of
3 heads) = 8 cores. Each core computes, for its (batch, head-group):
  - Q^T,K^T = Wqk @ x^T   (bf16 matmuls; bias folded into the PSUM->SBUF
              copy on the scalar engine: Identity activation + bias AP)
  - Vn      = x-chunk.T @ Wv-stream: V in NATURAL layout [keys, 3*(64+1)]
              per 128-key chunk (xT chunk stationary, V weights moving).
              The per-head ones column (softmax denominator) is a zero
              weight column plus 1.0 in the broadcast bias added on DVE.
  - S^T     = K^T.T-chunks vs Q^T   (keys on partitions, queries on free dim)
  - P~      = exp(0.125 * S^T)      (no max-subtraction; logits are tiny)
  - U^T     = Vn.T @ P~              (ones column gives denominator row)
  - O^T     = U^T * (1/den) broadcast (recip on DVE, partition-broadcast on
              GpSimd, multiply on DVE)
  - Z^T    += Wproj-slice @ O^T      (partial projection output, bf16)
Host sums the 4 head-group partials per batch and adds proj_b.

Schedule: K/Q chains first (nt-interleaved, DMA-paced), then the qk2 and
natural-V chains are interleaved with PRE-EMITTED S+exp pairs of the first
two (h0, qq) attention groups, so the scalar engine starts its ~65us of
exp work ~25us early.  Within every attention (head, qq) group, key-chunk
pairs are software-pipelined: S matmuls for pair k+1 are emitted before
the PV matmuls of pair k, keeping exp off the PE critical path.  Key
chunks are reordered so the first PV matmul of each group covers the full
512-query PSUM bank with start=True (pending-zero is bank-granular).  The
projection for quarter qq is emitted during the attention of quarter qq+1
so the output DMA overlaps compute; its casts run on DVE early and on the
scalar engine for the last two quarters (after exp drains).
"""

import functools

import ml_dtypes
import numpy as np

import concourse.bass as bass
import concourse.bacc as bacc_mod
import concourse.mybir as mybir
import concourse.tile as tile
from concourse.bass import ts

F32 = mybir.dt.float32
BF16 = mybir.dt.bfloat16

B, T, N, D = 2, 8, 256, 768
H, HD = 12, 64
L = T * N          # 2048
HPC = 3            # heads per core
NKC = L // 128     # 16 key chunks of 128
NDC = D // 128     # 6 contraction chunks
SCALE = 1.0 / 8.0
VW = 3 * 65        # natural-V row width: [v_h0 | 1] [v_h1 | 1] [v_h2 | 1]


def group_pairs(qq):
    """Key-chunk pairs for one (head, qq) group; masked pair last."""
    kcs = list(range(4 * qq + 2, 16)) + [4 * qq, 4 * qq + 1]
    return [(kcs[i], kcs[i + 1]) for i in range(0, len(kcs), 2)]


def build_nc():
    nc = bacc_mod.Bacc()

    xT_d = nc.declare_dram_parameter("xT", [D, L], BF16, isOutput=False)
    wqkT_d = nc.declare_dram_parameter("wqkT", [D, 384], BF16, isOutput=False)
    wvT3_d = nc.declare_dram_parameter("wvT3", [D, VW], BF16, isOutput=False)
    bqk_d = nc.declare_dram_parameter("bqk", [128, 4], F32, isOutput=False)
    bvb_d = nc.declare_dram_parameter("bvb", [128, VW], F32, isOutput=False)
    wprojT_d = nc.declare_dram_parameter("wprojT", [128, 1536], BF16, isOutput=False)
    zT_d = nc.declare_dram_parameter("zT", [D, L], BF16, isOutput=True)

    with tile.TileContext(nc) as tc:
        with (
            tc.tile_pool(name="persist", bufs=1) as pp,
            tc.tile_pool(name="ptile", bufs=18) as ppool,
            tc.tile_pool(name="zbuf", bufs=3) as zpool,
            tc.tile_pool(name="psum_st", bufs=2, space="PSUM") as pst,
            tc.tile_pool(name="psum_ot", bufs=2, space="PSUM") as pot,
            tc.tile_pool(name="psum_mc", bufs=2, space="PSUM") as pmc,
        ):
            # ---- persistent SBUF tensors ----
            wqkT = pp.tile([128, NDC, 384], BF16, tag="wqkT")
            wvT3 = pp.tile([128, NDC, VW], BF16, tag="wvT3")
            bqk = pp.tile([128, 4], F32, tag="bqk")
            bvb = pp.tile([128, VW], F32, tag="bvb")
            wprojT = pp.tile([128, 1536], BF16, tag="wprojT")
            qt = pp.tile([128, L], BF16, tag="qt")      # [q_h0 | q_h1]
            kt = pp.tile([128, L], BF16, tag="kt")      # [k_h0 | k_h1]
            qk2 = pp.tile([128, L], BF16, tag="qk2")    # [q_h2 | k_h2]
            kt2 = pp.tile([64, L], BF16, tag="kt2")     # k_h2 re-based to part 0
            vn = pp.tile([128, NKC, VW], BF16, tag="vn")
            otp = pp.tile([128, L], BF16, tag="otp")    # [o_h0 | o_h1]
            ots = pp.tile([64, L], BF16, tag="ots")     # [o_h2]
            bcast = pp.tile([64, 512], F32, tag="bcast")
            den = pp.tile([1, 512], F32, tag="den")

            qt_src = [qt[0:64, :], qt[64:128, :], qk2[0:64, :]]
            kt_src = [kt[0:64, :], kt[64:128, :], kt2[0:64, :]]
            ot_dst = [otp[0:64, :], otp[64:128, :], ots[0:64, :]]

            def dram_w(d, cols, c0, c1):
                return d[:, c0:c1].rearrange("(dc p) w -> p dc w", p=128)

            def emit_s_pair(h, qq, pi, a, b):
                """S matmuls + exp for one key-chunk pair; returns a PV job."""
                q_lo = qq * 512
                masked = a == 4 * qq
                seg = 256 if masked else 512
                st2 = pst.tile([128, 1024], F32, tag="st")
                nc.tensor.matmul(
                    st2[:, 0:seg],
                    kt_src[h][:, ts(a, 128)],
                    qt_src[h][:, q_lo : q_lo + seg],
                    start=True, stop=True,
                )
                nc.tensor.matmul(
                    st2[:, 512 : 512 + seg],
                    kt_src[h][:, ts(b, 128)],
                    qt_src[h][:, q_lo : q_lo + seg],
                    start=True, stop=True,
                )
                pt = ppool.tile([128, 1024], BF16, tag="pt")
                if masked:
                    for off in (0, 512):
                        nc.scalar.activation(
                            pt[:, off : off + 256],
                            st2[:, off : off + 256],
                            mybir.ActivationFunctionType.Exp,
                            scale=SCALE,
                        )
                else:
                    nc.scalar.activation(
                        pt[:, 0:1024],
                        st2[:, 0:1024],
                        mybir.ActivationFunctionType.Exp,
                        scale=SCALE,
                    )
                return (pi, a, b, pt, masked)

            def emit_pv(h, ot, job):
                pi, a, b, pt, masked = job
                va = vn[:, a, 65 * h : 65 * h + 65]
                vb = vn[:, b, 65 * h : 65 * h + 65]
                if not masked:
                    nc.tensor.matmul(
                        ot[0:65, 0:512], va, pt[:, 0:512],
                        start=(pi == 0), stop=False, skip_group_check=True,
                    )
                    nc.tensor.matmul(
                        ot[0:65, 0:512], vb, pt[:, 512:1024],
                        start=False, stop=False, skip_group_check=True,
                    )
                else:
                    nc.tensor.matmul(
                        ot[0:65, 0:256], va, pt[:, 0:256],
                        start=False, stop=False, skip_group_check=True,
                    )
                    nc.tensor.matmul(
                        ot[0:65, 0:256], vb, pt[:, 512:768],
                        start=False, stop=True, skip_group_check=True,
                    )

            # ---- input DMAs ----
            # sync: the phase-1 critical path; gpsimd: everything needed later.
            nc.sync.dma_start(out=wqkT[:, :, 128:256], in_=dram_w(wqkT_d, 384, 128, 256))
            with tc.tile_pool(name="xT", bufs=1) as xp:
                xT = xp.tile([128, NDC, L], BF16, tag="xT")
                nc.sync.dma_start(
                    out=xT[:, :, ts(0, 512)],
                    in_=xT_d[:, ts(0, 512)].rearrange("(dc p) w -> p dc w", p=128),
                )
                nc.gpsimd.dma_start(out=bqk[:], in_=bqk_d[:, :])
                nc.sync.dma_start(out=wqkT[:, :, 0:128], in_=dram_w(wqkT_d, 384, 0, 128))
                nc.sync.dma_start(out=wqkT[:, :, 256:384], in_=dram_w(wqkT_d, 384, 256, 384))
                for nt in range(1, 4):
                    nc.sync.dma_start(
                        out=xT[:, :, ts(nt, 512)],
                        in_=xT_d[:, ts(nt, 512)].rearrange(
                            "(dc p) w -> p dc w", p=128
                        ),
                    )
                nc.gpsimd.dma_start(
                    out=wvT3[:],
                    in_=wvT3_d[:, :].rearrange("(dc p) w -> p dc w", p=128),
                )
                nc.gpsimd.dma_start(out=bvb[:], in_=bvb_d[:, :])
                nc.gpsimd.dma_start(out=wprojT[:], in_=wprojT_d[:, :])
                nc.vector.memset(bcast[:], 1.0)

                # ---- phase 1a: K^T and Q^T chains, nt-interleaved ----
                def qk_chain(mc, dst, nt):
                    ps = pmc.tile([128, 512], F32, tag="qs")
                    for dc in range(NDC):
                        nc.tensor.matmul(
                            ps[:],
                            wqkT[:, dc, ts(mc, 128)],
                            xT[:, dc, ts(nt, 512)],
                            start=(dc == 0),
                            stop=(dc == NDC - 1),
                        )
                    nc.scalar.activation(
                        dst[:, ts(nt, 512)],
                        ps[:],
                        mybir.ActivationFunctionType.Identity,
                        bias=bqk[:, mc : mc + 1],
                    )

                for nt in range(4):
                    qk_chain(1, kt, nt)   # keys first: S needs all of kt
                    qk_chain(0, qt, nt)
                # Pre-warm the exp table after the Identity bias-copies.
                warm = zpool.tile([128, 32], F32, tag="warm")
                nc.vector.memset(warm[:], 0.0)
                nc.scalar.activation(
                    warm[:], warm[:], mybir.ActivationFunctionType.Exp
                )

                # ---- phase 1b: qk2 + natural-V chains, interleaved with the
                # pre-emitted S+exp pairs of groups (h0,q0) and (h0,q1) ----
                pre_jobs = {(0, 0): [], (0, 1): []}
                s_slots = [
                    (hq, pi, a, b)
                    for hq in ((0, 0), (0, 1))
                    for pi, (a, b) in enumerate(group_pairs(hq[1]))
                ]
                s_it = iter(s_slots)

                def emit_next_s():
                    slot = next(s_it, None)
                    if slot is not None:
                        hq, pi, a, b = slot
                        pre_jobs[hq].append(emit_s_pair(hq[0], hq[1], pi, a, b))

                for nt in range(4):
                    # qk2 chain (bias on DVE: keep scalar exp-only past warm)
                    ps = pmc.tile([128, 512], F32, tag="qs")
                    for dc in range(NDC):
                        nc.tensor.matmul(
                            ps[:],
                            wqkT[:, dc, 256:384],
                            xT[:, dc, ts(nt, 512)],
                            start=(dc == 0),
                            stop=(dc == NDC - 1),
                        )
                    nc.vector.tensor_scalar_add(
                        qk2[:, ts(nt, 512)], ps[:], bqk[:, 2:3]
                    )
                    if nt == 3:
                        # k_h2 re-base: partitions 64:128 -> 0:64
                        nc.gpsimd.dma_start(out=kt2[0:64, :], in_=qk2[64:128, :])
                    for kc in range(4 * nt, 4 * nt + 4):
                        vp = pot.tile([128, 256], F32, tag="ot")
                        for dc in range(NDC):
                            nc.tensor.matmul(
                                vp[:, 0:VW],
                                xT[:, dc, ts(kc, 128)],
                                wvT3[:, dc, :],
                                start=(dc == 0),
                                stop=(dc == NDC - 1),
                            )
                        nc.vector.tensor_tensor(
                            out=vn[:, kc, :],
                            in0=vp[:, 0:VW],
                            in1=bvb[:],
                            op=mybir.AluOpType.add,
                        )
                        emit_next_s()
                while next(s_it, None) is not None:
                    pass  # (all slots consumed above: 16 slots, 14 pairs)

            # ---- attention + interleaved projection ----
            def attn_group(h, qq):
                ot = pot.tile([128, 512], F32, tag="ot")
                jobs = pre_jobs.pop((h, qq), None)
                if jobs is not None:
                    for job in jobs:
                        emit_pv(h, ot, job)
                else:
                    pending = []
                    for pi, (a, b) in enumerate(group_pairs(qq)):
                        pending.append(emit_s_pair(h, qq, pi, a, b))
                        if len(pending) > 1:
                            emit_pv(h, ot, pending.pop(0))
                    emit_pv(h, ot, pending.pop(0))
                # normalize: inv = 1/den, broadcast across 64 partitions
                q_lo = qq * 512
                nc.vector.tensor_copy(den[0:1, :], ot[64:65, 0:512])
                nc.vector.reciprocal_approx_fast(bcast[0:1, :], den[0:1, :])
                nc.gpsimd.partition_broadcast(bcast[0:64, :], bcast[0:1, :])
                nc.vector.tensor_tensor(
                    out=ot_dst[h][:, q_lo : q_lo + 512],
                    in0=ot[0:64, 0:512],
                    in1=bcast[0:64, :],
                    op=mybir.AluOpType.mult,
                )

            def proj(qq, casts_on_scalar=False):
                for mc in range(NDC):
                    ps = pmc.tile([128, 512], F32, tag="qs")
                    nc.tensor.matmul(
                        ps[:],
                        wprojT[:, ts(mc, 128)],
                        otp[:, ts(qq, 512)],
                        start=True, stop=False,
                    )
                    nc.tensor.matmul(
                        ps[:],
                        wprojT[0:64, 768 + mc * 128 : 768 + (mc + 1) * 128],
                        ots[0:64, ts(qq, 512)],
                        start=False, stop=True,
                    )
                    zb = zpool.tile([128, 512], BF16, tag="zb")
                    if casts_on_scalar:
                        nc.scalar.copy(zb[:], ps[:])
                    else:
                        nc.vector.tensor_copy(zb[:], ps[:])
                    nc.sync.dma_start(
                        out=zT_d[ts(mc, 128), ts(qq, 512)], in_=zb[:]
                    )

            for qq in range(4):
                for h in range(HPC):
                    attn_group(h, qq)
                    if h == 0 and qq > 0:
                        proj(qq - 1, casts_on_scalar=(qq == 3))
            proj(3, casts_on_scalar=True)

    nc.compile()
    return nc


@functools.lru_cache(maxsize=1)
def get_nc():
    return build_nc()


def make_in_maps(x, qkv_w, qkv_b, proj_w):
    """Per-core host-side sharding/layout prep."""
    x = np.asarray(x, dtype=np.float32)
    qkv_w = np.asarray(qkv_w, dtype=np.float32)
    qkv_b = np.asarray(qkv_b, dtype=np.float32)
    proj_w = np.asarray(proj_w, dtype=np.float32)

    in_maps = []
    for c in range(8):
        b, g = divmod(c, 4)
        h0, h1, h2 = 3 * g, 3 * g + 1, 3 * g + 2

        def qrows(h):
            return slice(h * HD, (h + 1) * HD)

        def krows(h):
            return slice(D + h * HD, D + (h + 1) * HD)

        def vrows(h):
            return slice(2 * D + h * HD, 2 * D + (h + 1) * HD)

        # qk selection: mc0=[q0|q1] mc1=[k0|k1] mc2=[q2|k2]
        order = [
            qrows(h0), qrows(h1), krows(h0), krows(h1), qrows(h2), krows(h2),
        ]
        wqk = np.concatenate([qkv_w[s] for s in order], axis=0)       # (384, 768)
        bqk_sel = np.concatenate([qkv_b[s] for s in order], axis=0)   # (384,)
        bcol = np.zeros((128, 4), np.float32)
        for mc in range(3):
            bcol[:, mc] = bqk_sel[mc * 128 : (mc + 1) * 128]
        # natural-V weights: per head 64 cols + one zero col (ones slot)
        wv3 = np.zeros((VW, D), np.float32)
        bv3 = np.zeros(VW, np.float32)
        for i, h in enumerate((h0, h1, h2)):
            wv3[65 * i : 65 * i + 64] = qkv_w[vrows(h)]
            bv3[65 * i : 65 * i + 64] = qkv_b[vrows(h)]
            bv3[65 * i + 64] = 1.0
        wpp = np.concatenate(
            [proj_w[:, ts_np(h0)].T, proj_w[:, ts_np(h1)].T], axis=0
        )  # (128, 768)
        wps = np.concatenate(
            [proj_w[:, ts_np(h2)].T, np.zeros((64, D), np.float32)], axis=0
        )  # (128, 768)
        in_maps.append(
            {
                "xT": np.ascontiguousarray(x[b].reshape(L, D).T).astype(
                    ml_dtypes.bfloat16
                ),
                "wqkT": np.ascontiguousarray(wqk.T).astype(ml_dtypes.bfloat16),
                "wvT3": np.ascontiguousarray(wv3.T).astype(ml_dtypes.bfloat16),
                "bqk": bcol,
                "bvb": np.broadcast_to(bv3, (128, VW)).copy(),
                "wprojT": np.ascontiguousarray(
                    np.concatenate([wpp, wps], axis=1)
                ).astype(ml_dtypes.bfloat16),
            }
        )
    return in_maps


def ts_np(h):
    return slice(h * HD, (h + 1) * HD)


def assemble_output(results, proj_b):
    proj_b = np.asarray(proj_b, dtype=np.float32)
    out = np.zeros((B, L, D), np.float32)
    for c in range(8):
        b = c // 4
        out[b] += results[c]["zT"].astype(np.float32).T
    out += proj_b[None, None, :]
    return out.reshape(B, T, N, D)


def _install_ntff_hook():
    """The container's antenv stub lacks axon_hooks; recreate it from the
    boot helper so trace=True can profile through libaxon_pjrt."""
    import sys
    import types

    try:
        from antenv.axon_hooks import get_axon_ntff_profile_hook  # noqa: F401

        return
    except ImportError:
        pass
    import antenv
    from trn_agent_boot.trn_boot import _ntff_profile_via_ctypes

    state = {"hook": _ntff_profile_via_ctypes("/opt/axon/libaxon_pjrt.so")}
    mod = types.ModuleType("antenv.axon_hooks")
    mod.set_axon_ntff_profile_hook = lambda h: state.__setitem__("hook", h)
    mod.get_axon_ntff_profile_hook = lambda: state["hook"]
    sys.modules["antenv.axon_hooks"] = mod
    antenv.axon_hooks = mod

    import concourse.bass_utils as bu

    orig_upload = bu.upload_artifacts

    def safe_upload(tmpdir):
        try:
            return orig_upload(tmpdir)
        except Exception:
            return tmpdir

    bu.upload_artifacts = safe_upload


def kernel_with_stats(x, qkv_w, qkv_b, proj_w, proj_b, trace=False):
    from concourse.bass_utils import run_bass_kernel_spmd

    if trace:
        _install_ntff_hook()
    nc = get_nc()
    in_maps = make_in_maps(x, qkv_w, qkv_b, proj_w)
    res = run_bass_kernel_spmd(nc, in_maps, list(range(8)), trace=trace)
    return assemble_output(res.results, proj_b), res


def kernel(x, qkv_w, qkv_b, proj_w, proj_b):
    out, _ = kernel_with_stats(x, qkv_w, qkv_b, proj_w, proj_b)
    return out
